# revision 23
# baseline (speedup 1.0000x reference)
"""Trainium2 Bass kernel for nn_Entropy (histogram_binning): per-pixel Shannon
entropy of a 5x5-window KDE histogram over 256 intensity bins.

v4 design (sigmoid front end with per-image ACT-table phases):
  k(x,b) = sig'(10(x-b)) = v(1-v) with v = sigmoid(10(x-b)).
  Host ships xrep[i] = x - bl - 128*half in the padded block layout
  [5 lead | 8 x (4 pad + 96 w) | 2 trail] x 2 halves = 1614 cols (fp32:
  the bias trick evaluates sigma near t=8*sc, so t needs ~1e-3 ABSOLUTE
  accuracy at t up to 250 -- fp16 ulp there is 0.125 and wrecks the
  kernel taps; pads = +1e4 so sigmoid(pad)=1 -> k=0).  One tile per image serves all
  16 superchunks via the ACT bias: v_sc = Sigmoid(10*xrep - 80*sc).
  Pipeline per superchunk (8 bins x 2 halves x 96 w = 1536 useful):
    ACT   vt = Sigmoid(10*xrep - 80sc)     (1 op, 1614 cols, fp16)
    DVE   a5 = WIN5(vt)                    (custom scan: state += v(1-v)
                                            diffs -> exact SAME w-window)
    PE    4x band matmul -> q in PSUM      (const stationary band16)
    ACT   lt = Ln(q + 2e-6)                (1 op, [96,4,384] strided PSUM)
    DVE   et = q * lt                      (PSUM x fp16 -> fp16)
    Pool  acc += et                        (fp32 accumulator, 1 op)
  ACT table phases: all 16 sigmoids of image i run back-to-back
  (sigmoid table), then all 16 Lns (natural_log table) -> 2 table loads
  per image instead of 2 per superchunk.  A 16-deep a5 pool carries the
  front end's outputs across the phase boundary; PE/DVE/Pool pipeline
  freely across phases.
  Per image: T = sum over the 16 bin-positions of acc (one strided
  tensor_reduce) -> T = sum_b q ln q.  Analytic S path (3 sigmoid taps
  on frac = x - round(x)) gives S = sum_b q; E = ln(S+EPS) - T/(S+EPS).
  Sharding: B*C = 24 images, 3 per core across 8 cores; no collectives.
"""

import sys

sys.path.insert(0, "/opt/trn_rl_repo")

import numpy as np

H = 96
W = 96
NIMG = 3
NCORES = 8
EPS = 1e-10
EPS1 = 2e-6
NSC = 16
BPS = 8            # bins per superchunk per half
BLK = 100          # per-bin block: 4 pads + 96 w
HCH = 5 + BPS * BLK + 2   # half-chunk cols: 5 lead + 800 + 2 trail = 807
PAD = 1e4          # xrep pad: sigmoid(10*PAD - anything) == 1 -> k == 0

_CACHE = {}


def _register_dve_ops():
    import concourse.dve_ops as dve_ops
    from concourse.dve_ops import DveOp
    from concourse.dve_spec import AluOp, One, Spec, Src0, Src1, scan

    def register(op):
        if op.name not in dve_ops._SUB_OPCODE_FOR_NAME:
            dve_ops.OPS.append(op)
            dve_ops._SUB_OPCODE_FOR_NAME[op.name] = (
                dve_ops._CUSTOM_DVE_ROW_BASE + len(dve_ops.OPS) - 1
            )
        else:
            op = next(o for o in dve_ops.OPS if o.name == op.name)
        return op

    win5 = register(DveOp(
        "WIN5K_ANT",
        Spec(body=scan(AluOp.ADD, Src0 * (One - Src0) - Src1 * (One - Src1)),
             reference=lambda in0, in1, c0, c1, c2: np.cumsum(
                 in0.astype(np.float32) * (1 - in0.astype(np.float32))
                 - in1.astype(np.float32) * (1 - in1.astype(np.float32)),
                 axis=-1, dtype=np.float32)),
        subdim=False,
        perf_en={"v3": True, "v4": True},
        uops_sha={"v3": "9d91f28b1ae18abb", "v4": "1425a9f273284709"}))

    return win5


def _emit_kernel(nc, tc, ctx, ins, outs, win5):
    from concourse import mybir

    f32 = mybir.dt.float32
    f16 = mybir.dt.float16
    i32 = mybir.dt.int32
    AF = mybir.ActivationFunctionType
    OP = mybir.AluOpType

    x_d, xrep_d, band_d = ins
    (ent_d,) = outs
    NW = NIMG * W

    consts = ctx.enter_context(tc.tile_pool(name="consts", bufs=1))
    sm = ctx.enter_context(tc.tile_pool(name="sm", bufs=1))
    vpool = ctx.enter_context(tc.tile_pool(name="vp", bufs=3))
    a5pool = ctx.enter_context(tc.tile_pool(name="a5p", bufs=1))
    lpool = ctx.enter_context(tc.tile_pool(name="lp", bufs=3))
    epool = ctx.enter_context(tc.tile_pool(name="ep", bufs=6))
    ppsum = ctx.enter_context(tc.tile_pool(name="pps", bufs=4, space="PSUM"))

    # ---- constants / inputs ----
    band_sb = consts.tile([H, H], f16)
    xrep = consts.tile([H, NIMG, 2 * HCH], f32)
    nc.sync.dma_start(xrep[:, 0, 0:538], xrep_d[0][:, 0:538])
    nc.gpsimd.dma_start(xrep[:, 0, 538:1076], xrep_d[0][:, 538:1076])
    nc.sync.dma_start(xrep[:, 0, 1076:1614], xrep_d[0][:, 1076:1614])
    nc.gpsimd.dma_start(band_sb[:], band_d[:])
    xall = consts.tile([H, NW], f32)
    for i in range(NIMG):
        nc.gpsimd.dma_start(xall[:, i * W:(i + 1) * W], x_d[i])
    for i in range(1, NIMG):
        nc.sync.dma_start(xrep[:, i, :], xrep_d[i])

    bias_tiles = {}

    def bias_ap(val):
        if val not in bias_tiles:
            t = consts.tile([H, 1], f32, tag=f"bias{val}")
            nc.vector.memset(t[:], val)
            bias_tiles[val] = t
        return bias_tiles[val][:]

    acc0 = consts.tile([H, 1536], f32)
    acc1 = consts.tile([H, 1536], f32)
    acc2 = consts.tile([H, 1536], f32)
    accs = [acc0, acc1, acc2]
    QL = sm.tile([H, NW], f32)

    # S path tiles (ops are emitted inside image 0's phases)
    shp = sm.tile([H, NIMG, W + 4], f32)
    nc.vector.memset(shp[:], 0.0)
    swin = sm.tile([H, NIMG, W], f32)
    sw_flat = swin[:].rearrange("p a b -> p (a b)")
    rtile = sm.tile([H, NW], f32)
    lns = sm.tile([H, NW], f32)   # filled during image-0 ln phase

    def emit_spath_A():
        # frac = x - rint(x) (DVE, early: only needs the x DMA)
        ni = sm.tile([H, NW], i32)
        nc.vector.tensor_copy(ni[:], xall[:])
        nf = sm.tile([H, NW], f32)
        nc.vector.tensor_copy(nf[:], ni[:])
        frac = sm.tile([H, NW], f32)
        nc.vector.tensor_tensor(frac[:], xall[:], nf[:], op=OP.subtract)
        return nf, frac

    def emit_spath_B(frac):
        vtap = sm.tile([H, 3, NW], f32)
        for oi, o in enumerate((-1, 0, 1)):
            nc.scalar.activation(
                vtap[:, oi, :], frac[:], AF.Sigmoid,
                scale=10.0, bias=bias_ap(float(-10 * o)))
        return vtap

    def emit_spath_C(nf, vtap):
        ktap = sm.tile([H, 3, NW], f32)
        nc.vector.tensor_tensor(ktap[:], vtap[:], vtap[:], op=OP.mult)
        nc.vector.tensor_tensor(ktap[:], vtap[:], ktap[:], op=OP.subtract)
        spix = sm.tile([H, NW], f32)
        nc.vector.tensor_copy(spix[:], ktap[:, 1, :])
        for oi, o in ((0, -1), (2, 1)):
            m = sm.tile([H, NW], f32, tag=f"m{o}", name=f"m{o}")
            if o < 0:
                nc.vector.tensor_scalar(m[:], nf[:], 1.0, None, op0=OP.is_ge)
            else:
                nc.vector.tensor_scalar(m[:], nf[:], 254.0, None, op0=OP.is_le)
            nc.vector.tensor_tensor(m[:], m[:], ktap[:, oi, :], op=OP.mult)
            nc.vector.tensor_tensor(spix[:], spix[:], m[:], op=OP.add)
        spix16 = sm.tile([H, NW], f16)
        nc.vector.tensor_copy(spix16[:], spix[:])
        ps_s = ppsum.tile([H, 1024], f32, tag="pt")
        nc.tensor.matmul(
            ps_s[:, 0:NW], band_sb[:], spix16[:], start=True, stop=True)
        return ps_s

    def emit_spath_D(ps_s):
        sh = sm.tile([H, NW], f32)
        nc.scalar.copy(sh[:], ps_s[:, 0:NW])
        for ii in range(NIMG):
            nc.vector.tensor_copy(shp[:, ii, 2:2 + W], sh[:, ii * W:(ii + 1) * W])
        nc.vector.tensor_tensor(
            swin[:], shp[:, :, 0:W], shp[:, :, 1:1 + W], op=OP.add)
        for j in (2, 3, 4):
            nc.vector.tensor_tensor(swin[:], swin[:], shp[:, :, j:j + W], op=OP.add)
        nc.vector.tensor_scalar(rtile[:], sw_flat, EPS, None, op0=OP.add)
        nc.vector.reciprocal(rtile[:], rtile[:])

    # =====================  main loop  =====================
    # per-image sigmoid bias tiles; images >0 gate on the previous image's
    # last Ln output so the scheduler cannot interleave sigmoids (sigmoid
    # table) into the Ln phase (natural_log table) and thrash table loads
    sbias = []
    for i in range(NIMG):
        row = []
        for sc in range(NSC):
            sb_t = consts.tile([H, 1], f32, tag=f"sb{i}_{sc}", name=f"sb{i}_{sc}")
            row.append(sb_t)
        sbias.append(row)
    for sc in range(NSC):
        nc.vector.memset(sbias[0][sc][:], float(-80 * sc))
    last_lt = [None]

    def emit_reduce(i):
        racc = accs[i][:].rearrange("p (g w) -> p w g", g=16)
        nc.vector.tensor_reduce(
            QL[:, i * W:(i + 1) * W], racc,
            axis=mybir.AxisListType.X, op=OP.add)

    leafpool = ctx.enter_context(tc.tile_pool(name="leafp", bufs=1))
    et_hold = [None]

    def emit_combines(i, lvs):
        c0 = leafpool.tile([H, 1536], f16, tag="c0", name="c0")
        c1 = leafpool.tile([H, 1536], f16, tag="c1", name="c1")
        c2 = leafpool.tile([H, 1536], f16, tag="c2", name="c2")
        nc.vector.tensor_tensor(c0[:], lvs[0][:], lvs[1][:], op=OP.add)
        nc.vector.tensor_tensor(c1[:], lvs[2][:], lvs[3][:], op=OP.add)
        nc.vector.tensor_tensor(c2[:], lvs[4][:], lvs[5][:], op=OP.add)
        nc.vector.tensor_tensor(c0[:], c0[:], c1[:], op=OP.add)
        nc.vector.tensor_tensor(c1[:], lvs[6][:], lvs[7][:], op=OP.add)
        nc.vector.tensor_tensor(c1[:], c1[:], c2[:], op=OP.add)
        nc.vector.tensor_tensor(accs[i][:], c0[:], c1[:], op=OP.add)

    nf_frac = [None]
    vtap_h = [None]
    ps_s_h = [None]
    for i in range(NIMG):
        acc = accs[i]
        # ---- sigmoid phase: all 16 superchunks' front ends ----
        if i > 0:
            emit_combines(i - 1, prev_leaves)
            emit_reduce(i - 1)
        if i == 0:
            nf_frac[0] = emit_spath_A()
        a5s = []
        vt_last = None
        for sc in range(NSC):
            vt = vpool.tile([H, 2 * HCH], f16, tag="v")
            nc.scalar.activation(
                vt[:], xrep[:, i, :], AF.Sigmoid,
                scale=10.0, bias=sbias[i][sc][:])
            a5 = a5pool.tile([H, 2 * HCH], f16, tag=f"a5_{sc}")
            nc.vector._custom_dve(
                win5, out=a5[:, 0:2 * HCH - 5], in0=vt[:, 5:2 * HCH],
                in1=vt[:, 0:2 * HCH - 5])
            a5s.append(a5)
            vt_last = vt

        if i == 0:
            vtap_h[0] = emit_spath_B(nf_frac[0][1])
            ps_s_h[0] = emit_spath_C(nf_frac[0][0], vtap_h[0])
            emit_spath_D(ps_s_h[0])
        # gate this image's Ln phase on its own sigmoid phase (table order)
        ebias = consts.tile([H, 1], f32, tag=f"eb{i}", name=f"eb{i}")
        nc.vector.tensor_scalar(
            ebias[:], vt_last[:, 0:1], 0.0, EPS1, op0=OP.mult, op1=OP.add)

        leaves = []
        # ---- ln phase: half-superchunk granularity, 4-deep PSUM ----
        if i == 0:
            nc.scalar.activation(lns[:], sw_flat, AF.Ln, bias=bias_ap(EPS))

        def emit_band_h(j):
            sc, hf = j // 2, j % 2
            pt = ppsum.tile([H, 1024], f32, tag="pt")
            for pp in range(2):
                off = hf * HCH + 400 * pp + 6
                mv = a5s[sc][:, off:off + 400] \
                    .rearrange("p (b z) -> p b z", z=BLK)[:, :, 0:96]
                nc.tensor.matmul(
                    pt[:, 512 * pp:512 * pp + 384],
                    band_sb[:], mv, start=True, stop=True)
            return pt

        lt_cur = et_cur = None

        def emit_back_h(j, pt):
            nonlocal lt_cur, et_cur
            sc, hf = j // 2, j % 2
            if hf == 0:
                lt_cur = lpool.tile([H, 1536], f16, tag="l")
                et_cur = epool.tile([H, 1536], f16, tag="e")
            qv = pt[:].rearrange("p (a b) -> p a b", b=512)[:, :, 0:384]
            lv = lt_cur[:, hf * 768:(hf + 1) * 768] \
                .rearrange("p (a b) -> p a b", b=384)
            nc.scalar.activation(lv, qv, AF.Ln, bias=ebias[:])
            nc.vector.tensor_tensor(
                et_cur[:, hf * 768:(hf + 1) * 768]
                .rearrange("p (a b) -> p a b", b=384), qv, lv, op=OP.mult)
            if hf == 1:
                # Pool does only the 8 pair-leaves, inside the ln phase
                # (it cannot absorb a full 16-add chain and any spill into
                # the sigmoid phase halves both Pool and DVE throughput)
                if sc % 2 == 0:
                    et_hold[0] = et_cur
                else:
                    leaf = leafpool.tile([H, 1536], f16,
                                         tag=f"leaf{sc // 2}",
                                         name=f"leaf{sc // 2}")
                    nc.gpsimd.tensor_tensor(
                        leaf[:], et_hold[0][:], et_cur[:], op=OP.add)
                    leaves.append(leaf)
                if sc == NSC - 1 and i + 1 < NIMG:
                    last_lt[0] = lt_cur

        NH = 2 * NSC
        pts = {j: emit_band_h(j) for j in range(3)}
        for j in range(NH):
            if j + 3 < NH:
                pts[j + 3] = emit_band_h(j + 3)
            emit_back_h(j, pts.pop(j))

        if i + 1 < NIMG:
            for sc in range(NSC):
                nc.vector.tensor_scalar(
                    sbias[i + 1][sc][:], last_lt[0][:, 0:1], 0.0,
                    float(-80 * sc), op0=OP.mult, op1=OP.add)
        prev_leaves = leaves
    # Tail ops: manual scheduler floor so the list scheduler cannot park
    # them early in the in-order DVE queue.
    ctx.enter_context(tc.tile_wait_until(1.0))
    emit_combines(NIMG - 1, prev_leaves)
    emit_reduce(NIMG - 1)

    # E = lnS - r*T
    ent = sm.tile([H, NW], f32)
    nc.vector.tensor_tensor(ent[:], rtile[:], QL[:], op=OP.mult)
    nc.vector.tensor_tensor(ent[:], lns[:], ent[:], op=OP.subtract)
    for i in range(NIMG):
        nc.sync.dma_start(ent_d[i], ent[:, i * W:(i + 1) * W])


def _get_compiled():
    if "nc" in _CACHE:
        return _CACHE["nc"]
    from contextlib import ExitStack

    import concourse.tile as tile
    from concourse import bacc, mybir

    win5 = _register_dve_ops()

    f32 = mybir.dt.float32
    f16 = mybir.dt.float16
    nc = bacc.Bacc("TRN2", target_bir_lowering=False, debug=False)
    x_d = nc.dram_tensor("x_sh", [NIMG, H, W], f32, kind="ExternalInput").ap()
    xrep_d = nc.dram_tensor(
        "xrep", [NIMG, H, 2 * HCH], f32, kind="ExternalInput").ap()
    band_d = nc.dram_tensor("band16", [H, H], f16, kind="ExternalInput").ap()
    ent_d = nc.dram_tensor("ent", [NIMG, H, W], f32, kind="ExternalOutput").ap()

    with tile.TileContext(nc) as tc:
        with ExitStack() as ctx:
            _emit_kernel(nc, tc, ctx, (x_d, xrep_d, band_d), (ent_d,), win5)
    nc.compile()
    _CACHE["nc"] = nc
    return nc


def _build_xrep(imgs):
    """imgs: [N, 96, 96] f32 -> [N, 96, 1614] f32 padded x-b map."""
    n = imgs.shape[0]
    out = np.full((n, H, 2 * HCH), PAD, dtype=np.float32)
    # cols: half*807 + 5 + bl*100 + 4 + w  <-  x - bl - 128*half
    for half in range(2):
        for bl in range(BPS):
            c0 = half * HCH + 5 + bl * BLK + 4
            out[:, :, c0:c0 + W] = imgs - (bl + 128 * half)
    return out


def make_in_maps(x):
    """x: full [8, 3, 96, 96] -> list of 8 per-core input dicts."""
    x = np.ascontiguousarray(np.asarray(x, dtype=np.float32))
    imgs = x.reshape(NCORES * NIMG, H, W)
    hh = np.arange(H)
    band = (np.abs(hh[:, None] - hh[None, :]) <= 2).astype(np.float16)
    xrep = _build_xrep(imgs)
    in_maps = []
    for c in range(NCORES):
        sh = np.ascontiguousarray(imgs[c * NIMG:(c + 1) * NIMG])
        in_maps.append(
            {
                "x_sh": sh,
                "xrep": np.ascontiguousarray(xrep[c * NIMG:(c + 1) * NIMG]),
                "band16": band,
            }
        )
    return in_maps


def kernel(x):
    """Full inputs in, full outputs out. x: [8, 3, 96, 96] f32."""
    from concourse.bass_utils import run_bass_kernel_spmd

    nc = _get_compiled()
    in_maps = make_in_maps(x)
    res = run_bass_kernel_spmd(nc, in_maps, list(range(NCORES)))
    out = np.stack([res.results[c]["ent"] for c in range(NCORES)])
    return out.reshape(8, 3, H, W).astype(np.float32)


# revision 24
# speedup vs baseline: 1.0183x; 1.0183x over previous
"""Trainium2 Bass kernel for nn_Entropy (histogram_binning): per-pixel Shannon
entropy of a 5x5-window KDE histogram over 256 intensity bins.

v4 design (sigmoid front end with per-image ACT-table phases):
  k(x,b) = sig'(10(x-b)) = v(1-v) with v = sigmoid(10(x-b)).
  Host ships xrep[i] = x - bl - 128*half in the padded block layout
  [5 lead | 8 x (4 pad + 96 w) | 2 trail] x 2 halves = 1614 cols (fp32:
  the bias trick evaluates sigma near t=8*sc, so t needs ~1e-3 ABSOLUTE
  accuracy at t up to 250 -- fp16 ulp there is 0.125 and wrecks the
  kernel taps; pads = +1e4 so sigmoid(pad)=1 -> k=0).  One tile per image serves all
  16 superchunks via the ACT bias: v_sc = Sigmoid(10*xrep - 80*sc).
  Pipeline per superchunk (8 bins x 2 halves x 96 w = 1536 useful):
    ACT   vt = Sigmoid(10*xrep - 80sc)     (1 op, 1614 cols, fp16)
    DVE   a5 = WIN5(vt)                    (custom scan: state += v(1-v)
                                            diffs -> exact SAME w-window)
    PE    4x band matmul -> q in PSUM      (const stationary band16)
    ACT   lt = Ln(q + 2e-6)                (1 op, [96,4,384] strided PSUM)
    DVE   et = q * lt                      (PSUM x fp16 -> fp16)
    Pool  acc += et                        (fp32 accumulator, 1 op)
  ACT table phases: all 16 sigmoids of image i run back-to-back
  (sigmoid table), then all 16 Lns (natural_log table) -> 2 table loads
  per image instead of 2 per superchunk.  A 16-deep a5 pool carries the
  front end's outputs across the phase boundary; PE/DVE/Pool pipeline
  freely across phases.
  Per image: T = sum over the 16 bin-positions of acc (one strided
  tensor_reduce) -> T = sum_b q ln q.  Analytic S path (3 sigmoid taps
  on frac = x - round(x)) gives S = sum_b q; E = ln(S+EPS) - T/(S+EPS).
  Sharding: B*C = 24 images, 3 per core across 8 cores; no collectives.
"""

import sys

sys.path.insert(0, "/opt/trn_rl_repo")

import numpy as np

H = 96
W = 96
NIMG = 3
NCORES = 8
EPS = 1e-10
EPS1 = 2e-6
NSC = 16
BPS = 8            # bins per superchunk per half
BLK = 100          # per-bin block: 4 pads + 96 w
HCH = 5 + BPS * BLK + 2   # half-chunk cols: 5 lead + 800 + 2 trail = 807
PAD = 1e4          # xrep pad: sigmoid(10*PAD - anything) == 1 -> k == 0

_CACHE = {}


def _register_dve_ops():
    import concourse.dve_ops as dve_ops
    from concourse.dve_ops import DveOp
    from concourse.dve_spec import AluOp, One, Spec, Src0, Src1, scan

    def register(op):
        if op.name not in dve_ops._SUB_OPCODE_FOR_NAME:
            dve_ops.OPS.append(op)
            dve_ops._SUB_OPCODE_FOR_NAME[op.name] = (
                dve_ops._CUSTOM_DVE_ROW_BASE + len(dve_ops.OPS) - 1
            )
        else:
            op = next(o for o in dve_ops.OPS if o.name == op.name)
        return op

    win5 = register(DveOp(
        "WIN5K_ANT",
        Spec(body=scan(AluOp.ADD, Src0 * (One - Src0) - Src1 * (One - Src1)),
             reference=lambda in0, in1, c0, c1, c2: np.cumsum(
                 in0.astype(np.float32) * (1 - in0.astype(np.float32))
                 - in1.astype(np.float32) * (1 - in1.astype(np.float32)),
                 axis=-1, dtype=np.float32)),
        subdim=False,
        perf_en={"v3": True, "v4": True},
        uops_sha={"v3": "9d91f28b1ae18abb", "v4": "1425a9f273284709"}))

    return win5


def _emit_kernel(nc, tc, ctx, ins, outs, win5):
    from concourse import mybir

    f32 = mybir.dt.float32
    f16 = mybir.dt.float16
    i32 = mybir.dt.int32
    AF = mybir.ActivationFunctionType
    OP = mybir.AluOpType

    x_d, xrep_d, band_d = ins
    (ent_d,) = outs
    NW = NIMG * W

    consts = ctx.enter_context(tc.tile_pool(name="consts", bufs=1))
    sm = ctx.enter_context(tc.tile_pool(name="sm", bufs=1))
    vpool = ctx.enter_context(tc.tile_pool(name="vp", bufs=3))
    a5pool = ctx.enter_context(tc.tile_pool(name="a5p", bufs=1))
    lpool = ctx.enter_context(tc.tile_pool(name="lp", bufs=3))
    epool = ctx.enter_context(tc.tile_pool(name="ep", bufs=6))
    ppsum = ctx.enter_context(tc.tile_pool(name="pps", bufs=4, space="PSUM"))

    # ---- constants / inputs ----
    band_sb = consts.tile([H, H], f16)
    xrep = consts.tile([H, NIMG, 2 * HCH], f32)
    nc.gpsimd.dma_start(xrep[:, 0, 0:538], xrep_d[0][:, 0:538])
    nc.gpsimd.dma_start(xrep[:, 0, 538:1076], xrep_d[0][:, 538:1076])
    nc.gpsimd.dma_start(xrep[:, 0, 1076:1614], xrep_d[0][:, 1076:1614])
    nc.gpsimd.dma_start(band_sb[:], band_d[:])
    xall = consts.tile([H, NW], f32)
    for i in range(NIMG):
        nc.gpsimd.dma_start(xall[:, i * W:(i + 1) * W], x_d[i])
    for i in range(1, NIMG):
        nc.gpsimd.dma_start(xrep[:, i, :], xrep_d[i])

    bias_tiles = {}

    def bias_ap(val):
        if val not in bias_tiles:
            t = consts.tile([H, 1], f32, tag=f"bias{val}")
            nc.vector.memset(t[:], val)
            bias_tiles[val] = t
        return bias_tiles[val][:]

    acc0 = consts.tile([H, 1536], f32)
    acc1 = consts.tile([H, 1536], f32)
    acc2 = consts.tile([H, 1536], f32)
    accs = [acc0, acc1, acc2]
    QL = sm.tile([H, NW], f32)

    # S path tiles (ops are emitted inside image 0's phases)
    shp = sm.tile([H, NIMG, W + 4], f32)
    nc.vector.memset(shp[:], 0.0)
    swin = sm.tile([H, NIMG, W], f32)
    sw_flat = swin[:].rearrange("p a b -> p (a b)")
    rtile = sm.tile([H, NW], f32)
    lns = sm.tile([H, NW], f32)   # filled during image-0 ln phase

    def emit_spath_A():
        # frac = x - rint(x) (DVE, early: only needs the x DMA)
        ni = sm.tile([H, NW], i32)
        nc.vector.tensor_copy(ni[:], xall[:])
        nf = sm.tile([H, NW], f32)
        nc.vector.tensor_copy(nf[:], ni[:])
        frac = sm.tile([H, NW], f32)
        nc.vector.tensor_tensor(frac[:], xall[:], nf[:], op=OP.subtract)
        return nf, frac

    def emit_spath_B(frac):
        vtap = sm.tile([H, 3, NW], f32)
        for oi, o in enumerate((-1, 0, 1)):
            nc.scalar.activation(
                vtap[:, oi, :], frac[:], AF.Sigmoid,
                scale=10.0, bias=bias_ap(float(-10 * o)))
        return vtap

    def emit_spath_C(nf, vtap):
        ktap = sm.tile([H, 3, NW], f32)
        nc.vector.tensor_tensor(ktap[:], vtap[:], vtap[:], op=OP.mult)
        nc.vector.tensor_tensor(ktap[:], vtap[:], ktap[:], op=OP.subtract)
        spix = sm.tile([H, NW], f32)
        nc.vector.tensor_copy(spix[:], ktap[:, 1, :])
        for oi, o in ((0, -1), (2, 1)):
            m = sm.tile([H, NW], f32, tag=f"m{o}", name=f"m{o}")
            if o < 0:
                nc.vector.tensor_scalar(m[:], nf[:], 1.0, None, op0=OP.is_ge)
            else:
                nc.vector.tensor_scalar(m[:], nf[:], 254.0, None, op0=OP.is_le)
            nc.vector.tensor_tensor(m[:], m[:], ktap[:, oi, :], op=OP.mult)
            nc.vector.tensor_tensor(spix[:], spix[:], m[:], op=OP.add)
        spix16 = sm.tile([H, NW], f16)
        nc.vector.tensor_copy(spix16[:], spix[:])
        ps_s = ppsum.tile([H, 1024], f32, tag="pt")
        nc.tensor.matmul(
            ps_s[:, 0:NW], band_sb[:], spix16[:], start=True, stop=True)
        return ps_s

    def emit_spath_D(ps_s):
        sh = sm.tile([H, NW], f32)
        nc.scalar.copy(sh[:], ps_s[:, 0:NW])
        for ii in range(NIMG):
            nc.vector.tensor_copy(shp[:, ii, 2:2 + W], sh[:, ii * W:(ii + 1) * W])
        nc.vector.tensor_tensor(
            swin[:], shp[:, :, 0:W], shp[:, :, 1:1 + W], op=OP.add)
        for j in (2, 3, 4):
            nc.vector.tensor_tensor(swin[:], swin[:], shp[:, :, j:j + W], op=OP.add)
        nc.vector.tensor_scalar(rtile[:], sw_flat, EPS, None, op0=OP.add)
        nc.vector.reciprocal(rtile[:], rtile[:])

    # =====================  main loop  =====================
    # per-image sigmoid bias tiles; images >0 gate on the previous image's
    # last Ln output so the scheduler cannot interleave sigmoids (sigmoid
    # table) into the Ln phase (natural_log table) and thrash table loads
    sbias = []
    for i in range(NIMG):
        row = []
        for sc in range(NSC):
            sb_t = consts.tile([H, 1], f32, tag=f"sb{i}_{sc}", name=f"sb{i}_{sc}")
            row.append(sb_t)
        sbias.append(row)
    for sc in range(NSC):
        nc.vector.memset(sbias[0][sc][:], float(-80 * sc))
    last_lt = [None]

    def emit_reduce(i):
        racc = accs[i][:].rearrange("p (g w) -> p w g", g=16)
        nc.vector.tensor_reduce(
            QL[:, i * W:(i + 1) * W], racc,
            axis=mybir.AxisListType.X, op=OP.add)

    leafpool = ctx.enter_context(tc.tile_pool(name="leafp", bufs=1))
    et_hold = [None]

    def emit_combines(i, lvs):
        c0 = leafpool.tile([H, 1536], f16, tag="c0", name="c0")
        c1 = leafpool.tile([H, 1536], f16, tag="c1", name="c1")
        c2 = leafpool.tile([H, 1536], f16, tag="c2", name="c2")
        nc.vector.tensor_tensor(c0[:], lvs[0][:], lvs[1][:], op=OP.add)
        nc.vector.tensor_tensor(c1[:], lvs[2][:], lvs[3][:], op=OP.add)
        nc.vector.tensor_tensor(c2[:], lvs[4][:], lvs[5][:], op=OP.add)
        nc.vector.tensor_tensor(c0[:], c0[:], c1[:], op=OP.add)
        nc.vector.tensor_tensor(c1[:], lvs[6][:], lvs[7][:], op=OP.add)
        nc.vector.tensor_tensor(c1[:], c1[:], c2[:], op=OP.add)
        nc.vector.tensor_tensor(accs[i][:], c0[:], c1[:], op=OP.add)

    nf_frac = [None]
    vtap_h = [None]
    ps_s_h = [None]
    for i in range(NIMG):
        acc = accs[i]
        # ---- sigmoid phase: all 16 superchunks' front ends ----
        if i > 0:
            emit_combines(i - 1, prev_leaves)
            emit_reduce(i - 1)
        if i == 0:
            nf_frac[0] = emit_spath_A()
        a5s = []
        vt_last = None
        for sc in range(NSC):
            vt = vpool.tile([H, 2 * HCH], f16, tag="v")
            nc.scalar.activation(
                vt[:], xrep[:, i, :], AF.Sigmoid,
                scale=10.0, bias=sbias[i][sc][:])
            a5 = a5pool.tile([H, 2 * HCH], f16, tag=f"a5_{sc}")
            nc.vector._custom_dve(
                win5, out=a5[:, 0:2 * HCH - 5], in0=vt[:, 5:2 * HCH],
                in1=vt[:, 0:2 * HCH - 5])
            a5s.append(a5)
            vt_last = vt

        if i == 0:
            vtap_h[0] = emit_spath_B(nf_frac[0][1])
            ps_s_h[0] = emit_spath_C(nf_frac[0][0], vtap_h[0])
            emit_spath_D(ps_s_h[0])
        # gate this image's Ln phase on its own sigmoid phase (table order)
        ebias = consts.tile([H, 1], f32, tag=f"eb{i}", name=f"eb{i}")
        nc.vector.tensor_scalar(
            ebias[:], vt_last[:, 0:1], 0.0, EPS1, op0=OP.mult, op1=OP.add)

        leaves = []
        # ---- ln phase: half-superchunk granularity, 4-deep PSUM ----
        if i == 0:
            nc.scalar.activation(lns[:], sw_flat, AF.Ln, bias=bias_ap(EPS))

        def emit_band_h(j):
            sc, hf = j // 2, j % 2
            pt = ppsum.tile([H, 1024], f32, tag="pt")
            for pp in range(2):
                off = hf * HCH + 400 * pp + 6
                mv = a5s[sc][:, off:off + 400] \
                    .rearrange("p (b z) -> p b z", z=BLK)[:, :, 0:96]
                nc.tensor.matmul(
                    pt[:, 512 * pp:512 * pp + 384],
                    band_sb[:], mv, start=True, stop=True)
            return pt

        lt_cur = et_cur = None

        def emit_back_h(j, pt):
            nonlocal lt_cur, et_cur
            sc, hf = j // 2, j % 2
            if hf == 0:
                lt_cur = lpool.tile([H, 1536], f16, tag="l")
                et_cur = epool.tile([H, 1536], f16, tag="e")
            qv = pt[:].rearrange("p (a b) -> p a b", b=512)[:, :, 0:384]
            lv = lt_cur[:, hf * 768:(hf + 1) * 768] \
                .rearrange("p (a b) -> p a b", b=384)
            nc.scalar.activation(lv, qv, AF.Ln, bias=ebias[:])
            nc.vector.tensor_tensor(
                et_cur[:, hf * 768:(hf + 1) * 768]
                .rearrange("p (a b) -> p a b", b=384), qv, lv, op=OP.mult)
            if hf == 1:
                # Pool does only the 8 pair-leaves, inside the ln phase
                # (it cannot absorb a full 16-add chain and any spill into
                # the sigmoid phase halves both Pool and DVE throughput)
                if sc % 2 == 0:
                    et_hold[0] = et_cur
                else:
                    leaf = leafpool.tile([H, 1536], f16,
                                         tag=f"leaf{sc // 2}",
                                         name=f"leaf{sc // 2}")
                    nc.gpsimd.tensor_tensor(
                        leaf[:], et_hold[0][:], et_cur[:], op=OP.add)
                    leaves.append(leaf)
                if sc == NSC - 1 and i + 1 < NIMG:
                    last_lt[0] = lt_cur

        NH = 2 * NSC
        pts = {j: emit_band_h(j) for j in range(3)}
        for j in range(NH):
            if j + 3 < NH:
                pts[j + 3] = emit_band_h(j + 3)
            emit_back_h(j, pts.pop(j))

        if i + 1 < NIMG:
            for sc in range(NSC):
                nc.vector.tensor_scalar(
                    sbias[i + 1][sc][:], last_lt[0][:, 0:1], 0.0,
                    float(-80 * sc), op0=OP.mult, op1=OP.add)
        prev_leaves = leaves
    # Tail ops: manual scheduler floor so the list scheduler cannot park
    # them early in the in-order DVE queue.
    ctx.enter_context(tc.tile_wait_until(1.0))
    emit_combines(NIMG - 1, prev_leaves)
    emit_reduce(NIMG - 1)

    # E = lnS - r*T
    ent = sm.tile([H, NW], f32)
    nc.vector.tensor_tensor(ent[:], rtile[:], QL[:], op=OP.mult)
    nc.vector.tensor_tensor(ent[:], lns[:], ent[:], op=OP.subtract)
    for i in range(NIMG):
        nc.sync.dma_start(ent_d[i], ent[:, i * W:(i + 1) * W])


def _get_compiled():
    if "nc" in _CACHE:
        return _CACHE["nc"]
    from contextlib import ExitStack

    import concourse.tile as tile
    from concourse import bacc, mybir

    win5 = _register_dve_ops()

    f32 = mybir.dt.float32
    f16 = mybir.dt.float16
    nc = bacc.Bacc("TRN2", target_bir_lowering=False, debug=False)
    x_d = nc.dram_tensor("x_sh", [NIMG, H, W], f32, kind="ExternalInput").ap()
    xrep_d = nc.dram_tensor(
        "xrep", [NIMG, H, 2 * HCH], f32, kind="ExternalInput").ap()
    band_d = nc.dram_tensor("band16", [H, H], f16, kind="ExternalInput").ap()
    ent_d = nc.dram_tensor("ent", [NIMG, H, W], f32, kind="ExternalOutput").ap()

    with tile.TileContext(nc) as tc:
        with ExitStack() as ctx:
            _emit_kernel(nc, tc, ctx, (x_d, xrep_d, band_d), (ent_d,), win5)
    nc.compile()
    _CACHE["nc"] = nc
    return nc


def _build_xrep(imgs):
    """imgs: [N, 96, 96] f32 -> [N, 96, 1614] f32 padded x-b map."""
    n = imgs.shape[0]
    out = np.full((n, H, 2 * HCH), PAD, dtype=np.float32)
    # cols: half*807 + 5 + bl*100 + 4 + w  <-  x - bl - 128*half
    for half in range(2):
        for bl in range(BPS):
            c0 = half * HCH + 5 + bl * BLK + 4
            out[:, :, c0:c0 + W] = imgs - (bl + 128 * half)
    return out


def make_in_maps(x):
    """x: full [8, 3, 96, 96] -> list of 8 per-core input dicts."""
    x = np.ascontiguousarray(np.asarray(x, dtype=np.float32))
    imgs = x.reshape(NCORES * NIMG, H, W)
    hh = np.arange(H)
    band = (np.abs(hh[:, None] - hh[None, :]) <= 2).astype(np.float16)
    xrep = _build_xrep(imgs)
    in_maps = []
    for c in range(NCORES):
        sh = np.ascontiguousarray(imgs[c * NIMG:(c + 1) * NIMG])
        in_maps.append(
            {
                "x_sh": sh,
                "xrep": np.ascontiguousarray(xrep[c * NIMG:(c + 1) * NIMG]),
                "band16": band,
            }
        )
    return in_maps


def kernel(x):
    """Full inputs in, full outputs out. x: [8, 3, 96, 96] f32."""
    from concourse.bass_utils import run_bass_kernel_spmd

    nc = _get_compiled()
    in_maps = make_in_maps(x)
    res = run_bass_kernel_spmd(nc, in_maps, list(range(NCORES)))
    out = np.stack([res.results[c]["ent"] for c in range(NCORES)])
    return out.reshape(8, 3, H, W).astype(np.float32)


# revision 25
# speedup vs baseline: 1.0209x; 1.0025x over previous
"""Trainium2 Bass kernel for nn_Entropy (histogram_binning): per-pixel Shannon
entropy of a 5x5-window KDE histogram over 256 intensity bins.

v4 design (sigmoid front end with per-image ACT-table phases):
  k(x,b) = sig'(10(x-b)) = v(1-v) with v = sigmoid(10(x-b)).
  Host ships xrep[i] = x - bl - 128*half in the padded block layout
  [5 lead | 8 x (4 pad + 96 w) | 2 trail] x 2 halves = 1614 cols (fp32:
  the bias trick evaluates sigma near t=8*sc, so t needs ~1e-3 ABSOLUTE
  accuracy at t up to 250 -- fp16 ulp there is 0.125 and wrecks the
  kernel taps; pads = +1e4 so sigmoid(pad)=1 -> k=0).  One tile per image serves all
  16 superchunks via the ACT bias: v_sc = Sigmoid(10*xrep - 80*sc).
  Pipeline per superchunk (8 bins x 2 halves x 96 w = 1536 useful):
    ACT   vt = Sigmoid(10*xrep - 80sc)     (1 op, 1614 cols, fp16)
    DVE   a5 = WIN5(vt)                    (custom scan: state += v(1-v)
                                            diffs -> exact SAME w-window)
    PE    4x band matmul -> q in PSUM      (const stationary band16)
    ACT   lt = Ln(q + 2e-6)                (1 op, [96,4,384] strided PSUM)
    DVE   et = q * lt                      (PSUM x fp16 -> fp16)
    Pool  acc += et                        (fp32 accumulator, 1 op)
  ACT table phases: all 16 sigmoids of image i run back-to-back
  (sigmoid table), then all 16 Lns (natural_log table) -> 2 table loads
  per image instead of 2 per superchunk.  A 16-deep a5 pool carries the
  front end's outputs across the phase boundary; PE/DVE/Pool pipeline
  freely across phases.
  Per image: T = sum over the 16 bin-positions of acc (one strided
  tensor_reduce) -> T = sum_b q ln q.  Analytic S path (3 sigmoid taps
  on frac = x - round(x)) gives S = sum_b q; E = ln(S+EPS) - T/(S+EPS).
  Sharding: B*C = 24 images, 3 per core across 8 cores; no collectives.
"""

import sys

sys.path.insert(0, "/opt/trn_rl_repo")

import numpy as np

H = 96
W = 96
NIMG = 3
NCORES = 8
EPS = 1e-10
EPS1 = 2e-6
NSC = 16
BPS = 8            # bins per superchunk per half
BLK = 100          # per-bin block: 4 pads + 96 w
HCH = 5 + BPS * BLK + 2   # half-chunk cols: 5 lead + 800 + 2 trail = 807
PAD = 1e4          # xrep pad: sigmoid(10*PAD - anything) == 1 -> k == 0

_CACHE = {}


def _register_dve_ops():
    import concourse.dve_ops as dve_ops
    from concourse.dve_ops import DveOp
    from concourse.dve_spec import AluOp, One, Spec, Src0, Src1, scan

    def register(op):
        if op.name not in dve_ops._SUB_OPCODE_FOR_NAME:
            dve_ops.OPS.append(op)
            dve_ops._SUB_OPCODE_FOR_NAME[op.name] = (
                dve_ops._CUSTOM_DVE_ROW_BASE + len(dve_ops.OPS) - 1
            )
        else:
            op = next(o for o in dve_ops.OPS if o.name == op.name)
        return op

    win5 = register(DveOp(
        "WIN5K_ANT",
        Spec(body=scan(AluOp.ADD, Src0 * (One - Src0) - Src1 * (One - Src1)),
             reference=lambda in0, in1, c0, c1, c2: np.cumsum(
                 in0.astype(np.float32) * (1 - in0.astype(np.float32))
                 - in1.astype(np.float32) * (1 - in1.astype(np.float32)),
                 axis=-1, dtype=np.float32)),
        subdim=False,
        perf_en={"v3": True, "v4": True},
        uops_sha={"v3": "9d91f28b1ae18abb", "v4": "1425a9f273284709"}))

    return win5


def _emit_kernel(nc, tc, ctx, ins, outs, win5):
    from concourse import mybir

    f32 = mybir.dt.float32
    f16 = mybir.dt.float16
    i32 = mybir.dt.int32
    AF = mybir.ActivationFunctionType
    OP = mybir.AluOpType

    x_d, xrep_d, band_d = ins
    (ent_d,) = outs
    NW = NIMG * W

    consts = ctx.enter_context(tc.tile_pool(name="consts", bufs=1))
    sm = ctx.enter_context(tc.tile_pool(name="sm", bufs=1))
    vpool = ctx.enter_context(tc.tile_pool(name="vp", bufs=5))
    a5pool = ctx.enter_context(tc.tile_pool(name="a5p", bufs=1))
    lpool = ctx.enter_context(tc.tile_pool(name="lp", bufs=3))
    epool = ctx.enter_context(tc.tile_pool(name="ep", bufs=6))
    ppsum = ctx.enter_context(tc.tile_pool(name="pps", bufs=4, space="PSUM"))

    # ---- constants / inputs ----
    band_sb = consts.tile([H, H], f16)
    xrep = consts.tile([H, NIMG, 2 * HCH], f32)
    nc.gpsimd.dma_start(xrep[:, 0, 0:538], xrep_d[0][:, 0:538])
    nc.gpsimd.dma_start(xrep[:, 0, 538:1076], xrep_d[0][:, 538:1076])
    nc.gpsimd.dma_start(xrep[:, 0, 1076:1614], xrep_d[0][:, 1076:1614])
    nc.gpsimd.dma_start(band_sb[:], band_d[:])
    xall = consts.tile([H, NW], f32)
    for i in range(NIMG):
        nc.gpsimd.dma_start(xall[:, i * W:(i + 1) * W], x_d[i])
    for i in range(1, NIMG):
        nc.gpsimd.dma_start(xrep[:, i, :], xrep_d[i])

    bias_tiles = {}

    def bias_ap(val):
        if val not in bias_tiles:
            t = consts.tile([H, 1], f32, tag=f"bias{val}")
            nc.vector.memset(t[:], val)
            bias_tiles[val] = t
        return bias_tiles[val][:]

    acc0 = consts.tile([H, 1536], f32)
    acc1 = consts.tile([H, 1536], f32)
    acc2 = consts.tile([H, 1536], f32)
    accs = [acc0, acc1, acc2]
    QL = sm.tile([H, NW], f32)

    # S path tiles (ops are emitted inside image 0's phases)
    shp = sm.tile([H, NIMG, W + 4], f32)
    nc.vector.memset(shp[:], 0.0)
    swin = sm.tile([H, NIMG, W], f32)
    sw_flat = swin[:].rearrange("p a b -> p (a b)")
    rtile = sm.tile([H, NW], f32)
    lns = sm.tile([H, NW], f32)   # filled during image-0 ln phase

    def emit_spath_A():
        # frac = x - rint(x) (DVE, early: only needs the x DMA)
        ni = sm.tile([H, NW], i32)
        nc.vector.tensor_copy(ni[:], xall[:])
        nf = sm.tile([H, NW], f32)
        nc.vector.tensor_copy(nf[:], ni[:])
        frac = sm.tile([H, NW], f32)
        nc.vector.tensor_tensor(frac[:], xall[:], nf[:], op=OP.subtract)
        return nf, frac

    def emit_spath_B(frac):
        vtap = sm.tile([H, 3, NW], f32)
        for oi, o in enumerate((-1, 0, 1)):
            nc.scalar.activation(
                vtap[:, oi, :], frac[:], AF.Sigmoid,
                scale=10.0, bias=bias_ap(float(-10 * o)))
        return vtap

    def emit_spath_C(nf, vtap):
        ktap = sm.tile([H, 3, NW], f32)
        nc.vector.tensor_tensor(ktap[:], vtap[:], vtap[:], op=OP.mult)
        nc.vector.tensor_tensor(ktap[:], vtap[:], ktap[:], op=OP.subtract)
        spix = sm.tile([H, NW], f32)
        nc.vector.tensor_copy(spix[:], ktap[:, 1, :])
        for oi, o in ((0, -1), (2, 1)):
            m = sm.tile([H, NW], f32, tag=f"m{o}", name=f"m{o}")
            if o < 0:
                nc.vector.tensor_scalar(m[:], nf[:], 1.0, None, op0=OP.is_ge)
            else:
                nc.vector.tensor_scalar(m[:], nf[:], 254.0, None, op0=OP.is_le)
            nc.vector.tensor_tensor(m[:], m[:], ktap[:, oi, :], op=OP.mult)
            nc.vector.tensor_tensor(spix[:], spix[:], m[:], op=OP.add)
        spix16 = sm.tile([H, NW], f16)
        nc.vector.tensor_copy(spix16[:], spix[:])
        ps_s = ppsum.tile([H, 1024], f32, tag="pt")
        nc.tensor.matmul(
            ps_s[:, 0:NW], band_sb[:], spix16[:], start=True, stop=True)
        return ps_s

    def emit_spath_D(ps_s):
        sh = sm.tile([H, NW], f32)
        nc.scalar.copy(sh[:], ps_s[:, 0:NW])
        for ii in range(NIMG):
            nc.vector.tensor_copy(shp[:, ii, 2:2 + W], sh[:, ii * W:(ii + 1) * W])
        nc.vector.tensor_tensor(
            swin[:], shp[:, :, 0:W], shp[:, :, 1:1 + W], op=OP.add)
        for j in (2, 3, 4):
            nc.vector.tensor_tensor(swin[:], swin[:], shp[:, :, j:j + W], op=OP.add)
        nc.vector.tensor_scalar(rtile[:], sw_flat, EPS, None, op0=OP.add)
        nc.vector.reciprocal(rtile[:], rtile[:])

    # =====================  main loop  =====================
    # per-image sigmoid bias tiles; images >0 gate on the previous image's
    # last Ln output so the scheduler cannot interleave sigmoids (sigmoid
    # table) into the Ln phase (natural_log table) and thrash table loads
    sbias = []
    for i in range(NIMG):
        row = []
        for sc in range(NSC):
            sb_t = consts.tile([H, 1], f32, tag=f"sb{i}_{sc}", name=f"sb{i}_{sc}")
            row.append(sb_t)
        sbias.append(row)
    for sc in range(NSC):
        nc.vector.memset(sbias[0][sc][:], float(-80 * sc))
    last_lt = [None]

    def emit_reduce(i):
        racc = accs[i][:].rearrange("p (g w) -> p w g", g=16)
        nc.vector.tensor_reduce(
            QL[:, i * W:(i + 1) * W], racc,
            axis=mybir.AxisListType.X, op=OP.add)

    leafpool = ctx.enter_context(tc.tile_pool(name="leafp", bufs=1))
    et_hold = [None]

    def emit_combines(i, lvs):
        c0 = leafpool.tile([H, 1536], f16, tag="c0", name="c0")
        c1 = leafpool.tile([H, 1536], f16, tag="c1", name="c1")
        c2 = leafpool.tile([H, 1536], f16, tag="c2", name="c2")
        nc.vector.tensor_tensor(c0[:], lvs[0][:], lvs[1][:], op=OP.add)
        nc.vector.tensor_tensor(c1[:], lvs[2][:], lvs[3][:], op=OP.add)
        nc.vector.tensor_tensor(c2[:], lvs[4][:], lvs[5][:], op=OP.add)
        nc.vector.tensor_tensor(c0[:], c0[:], c1[:], op=OP.add)
        nc.vector.tensor_tensor(c1[:], lvs[6][:], lvs[7][:], op=OP.add)
        nc.vector.tensor_tensor(c1[:], c1[:], c2[:], op=OP.add)
        nc.vector.tensor_tensor(accs[i][:], c0[:], c1[:], op=OP.add)

    nf_frac = [None]
    vtap_h = [None]
    ps_s_h = [None]
    for i in range(NIMG):
        acc = accs[i]
        # ---- sigmoid phase: all 16 superchunks' front ends ----
        if i > 0:
            emit_combines(i - 1, prev_leaves)
            emit_reduce(i - 1)
        if i == 0:
            nf_frac[0] = emit_spath_A()
        a5s = []
        vt_last = None
        for sc in range(NSC):
            vt = vpool.tile([H, 2 * HCH], f16, tag="v")
            nc.scalar.activation(
                vt[:], xrep[:, i, :], AF.Sigmoid,
                scale=10.0, bias=sbias[i][sc][:])
            a5 = a5pool.tile([H, 2 * HCH], f16, tag=f"a5_{sc}")
            nc.vector._custom_dve(
                win5, out=a5[:, 0:2 * HCH - 5], in0=vt[:, 5:2 * HCH],
                in1=vt[:, 0:2 * HCH - 5])
            a5s.append(a5)
            vt_last = vt

        if i == 0:
            vtap_h[0] = emit_spath_B(nf_frac[0][1])
            ps_s_h[0] = emit_spath_C(nf_frac[0][0], vtap_h[0])
            emit_spath_D(ps_s_h[0])
        # gate this image's Ln phase on its own sigmoid phase (table order)
        ebias = consts.tile([H, 1], f32, tag=f"eb{i}", name=f"eb{i}")
        nc.vector.tensor_scalar(
            ebias[:], vt_last[:, 0:1], 0.0, EPS1, op0=OP.mult, op1=OP.add)

        leaves = []
        # ---- ln phase: half-superchunk granularity, 4-deep PSUM ----
        if i == 0:
            nc.scalar.activation(lns[:], sw_flat, AF.Ln, bias=bias_ap(EPS))

        def emit_band_h(j):
            sc, hf = j // 2, j % 2
            pt = ppsum.tile([H, 1024], f32, tag="pt")
            for pp in range(2):
                off = hf * HCH + 400 * pp + 6
                mv = a5s[sc][:, off:off + 400] \
                    .rearrange("p (b z) -> p b z", z=BLK)[:, :, 0:96]
                nc.tensor.matmul(
                    pt[:, 512 * pp:512 * pp + 384],
                    band_sb[:], mv, start=True, stop=True)
            return pt

        lt_cur = et_cur = None

        def emit_back_h(j, pt):
            nonlocal lt_cur, et_cur
            sc, hf = j // 2, j % 2
            if hf == 0:
                lt_cur = lpool.tile([H, 1536], f16, tag="l")
                et_cur = epool.tile([H, 1536], f16, tag="e")
            qv = pt[:].rearrange("p (a b) -> p a b", b=512)[:, :, 0:384]
            lv = lt_cur[:, hf * 768:(hf + 1) * 768] \
                .rearrange("p (a b) -> p a b", b=384)
            nc.scalar.activation(lv, qv, AF.Ln, bias=ebias[:])
            nc.vector.tensor_tensor(
                et_cur[:, hf * 768:(hf + 1) * 768]
                .rearrange("p (a b) -> p a b", b=384), qv, lv, op=OP.mult)
            if hf == 1:
                # Pool does only the 8 pair-leaves, inside the ln phase
                # (it cannot absorb a full 16-add chain and any spill into
                # the sigmoid phase halves both Pool and DVE throughput)
                if sc % 2 == 0:
                    et_hold[0] = et_cur
                else:
                    leaf = leafpool.tile([H, 1536], f16,
                                         tag=f"leaf{sc // 2}",
                                         name=f"leaf{sc // 2}")
                    nc.gpsimd.tensor_tensor(
                        leaf[:], et_hold[0][:], et_cur[:], op=OP.add)
                    leaves.append(leaf)
                if sc == NSC - 1 and i + 1 < NIMG:
                    last_lt[0] = lt_cur

        NH = 2 * NSC
        pts = {j: emit_band_h(j) for j in range(3)}
        for j in range(NH):
            if j + 3 < NH:
                pts[j + 3] = emit_band_h(j + 3)
            emit_back_h(j, pts.pop(j))

        if i + 1 < NIMG:
            for sc in range(NSC):
                nc.vector.tensor_scalar(
                    sbias[i + 1][sc][:], last_lt[0][:, 0:1], 0.0,
                    float(-80 * sc), op0=OP.mult, op1=OP.add)
        prev_leaves = leaves
    # Tail ops: manual scheduler floor so the list scheduler cannot park
    # them early in the in-order DVE queue.
    ctx.enter_context(tc.tile_wait_until(1.0))
    emit_combines(NIMG - 1, prev_leaves)
    emit_reduce(NIMG - 1)

    # E = lnS - r*T
    ent = sm.tile([H, NW], f32)
    nc.vector.tensor_tensor(ent[:], rtile[:], QL[:], op=OP.mult)
    nc.vector.tensor_tensor(ent[:], lns[:], ent[:], op=OP.subtract)
    for i in range(NIMG):
        nc.sync.dma_start(ent_d[i], ent[:, i * W:(i + 1) * W])


def _get_compiled():
    if "nc" in _CACHE:
        return _CACHE["nc"]
    from contextlib import ExitStack

    import concourse.tile as tile
    from concourse import bacc, mybir

    win5 = _register_dve_ops()

    f32 = mybir.dt.float32
    f16 = mybir.dt.float16
    nc = bacc.Bacc("TRN2", target_bir_lowering=False, debug=False)
    x_d = nc.dram_tensor("x_sh", [NIMG, H, W], f32, kind="ExternalInput").ap()
    xrep_d = nc.dram_tensor(
        "xrep", [NIMG, H, 2 * HCH], f32, kind="ExternalInput").ap()
    band_d = nc.dram_tensor("band16", [H, H], f16, kind="ExternalInput").ap()
    ent_d = nc.dram_tensor("ent", [NIMG, H, W], f32, kind="ExternalOutput").ap()

    with tile.TileContext(nc) as tc:
        with ExitStack() as ctx:
            _emit_kernel(nc, tc, ctx, (x_d, xrep_d, band_d), (ent_d,), win5)
    nc.compile()
    _CACHE["nc"] = nc
    return nc


def _build_xrep(imgs):
    """imgs: [N, 96, 96] f32 -> [N, 96, 1614] f32 padded x-b map."""
    n = imgs.shape[0]
    out = np.full((n, H, 2 * HCH), PAD, dtype=np.float32)
    # cols: half*807 + 5 + bl*100 + 4 + w  <-  x - bl - 128*half
    for half in range(2):
        for bl in range(BPS):
            c0 = half * HCH + 5 + bl * BLK + 4
            out[:, :, c0:c0 + W] = imgs - (bl + 128 * half)
    return out


def make_in_maps(x):
    """x: full [8, 3, 96, 96] -> list of 8 per-core input dicts."""
    x = np.ascontiguousarray(np.asarray(x, dtype=np.float32))
    imgs = x.reshape(NCORES * NIMG, H, W)
    hh = np.arange(H)
    band = (np.abs(hh[:, None] - hh[None, :]) <= 2).astype(np.float16)
    xrep = _build_xrep(imgs)
    in_maps = []
    for c in range(NCORES):
        sh = np.ascontiguousarray(imgs[c * NIMG:(c + 1) * NIMG])
        in_maps.append(
            {
                "x_sh": sh,
                "xrep": np.ascontiguousarray(xrep[c * NIMG:(c + 1) * NIMG]),
                "band16": band,
            }
        )
    return in_maps


def kernel(x):
    """Full inputs in, full outputs out. x: [8, 3, 96, 96] f32."""
    from concourse.bass_utils import run_bass_kernel_spmd

    nc = _get_compiled()
    in_maps = make_in_maps(x)
    res = run_bass_kernel_spmd(nc, in_maps, list(range(NCORES)))
    out = np.stack([res.results[c]["ent"] for c in range(NCORES)])
    return out.reshape(8, 3, H, W).astype(np.float32)


# revision 26
# speedup vs baseline: 1.1034x; 1.0808x over previous
"""Trainium2 Bass kernel for nn_Entropy (histogram_binning): per-pixel Shannon
entropy of a 5x5-window KDE histogram over 256 intensity bins.

v4 design (sigmoid front end with per-image ACT-table phases):
  k(x,b) = sig'(10(x-b)) = v(1-v) with v = sigmoid(10(x-b)).
  Host ships xrep[i] = x - bl - 128*half in the padded block layout
  [5 lead | 8 x (4 pad + 96 w) | 2 trail] x 2 halves = 1614 cols (fp32:
  the bias trick evaluates sigma near t=8*sc, so t needs ~1e-3 ABSOLUTE
  accuracy at t up to 250 -- fp16 ulp there is 0.125 and wrecks the
  kernel taps; pads = +1e4 so sigmoid(pad)=1 -> k=0).  One tile per image serves all
  16 superchunks via the ACT bias: v_sc = Sigmoid(10*xrep - 80*sc).
  Pipeline per superchunk (8 bins x 2 halves x 96 w = 1536 useful):
    ACT   vt = Sigmoid(10*xrep - 80sc)     (1 op, 1614 cols, fp16)
    DVE   a5 = WIN5(vt)                    (custom scan: state += v(1-v)
                                            diffs -> exact SAME w-window)
    PE    4x band matmul -> q in PSUM      (const stationary band16)
    ACT   lt = Ln(q + 2e-6)                (1 op, [96,4,384] strided PSUM)
    DVE   et = q * lt                      (PSUM x fp16 -> fp16)
    Pool  acc += et                        (fp32 accumulator, 1 op)
  ACT table phases: all 16 sigmoids of image i run back-to-back
  (sigmoid table), then all 16 Lns (natural_log table) -> 2 table loads
  per image instead of 2 per superchunk.  A 16-deep a5 pool carries the
  front end's outputs across the phase boundary; PE/DVE/Pool pipeline
  freely across phases.
  Per image: T = sum over the 16 bin-positions of acc (one strided
  tensor_reduce) -> T = sum_b q ln q.  Analytic S path (3 sigmoid taps
  on frac = x - round(x)) gives S = sum_b q; E = ln(S+EPS) - T/(S+EPS).
  Sharding: B*C = 24 images, 3 per core across 8 cores; no collectives.
"""

import sys

sys.path.insert(0, "/opt/trn_rl_repo")

import numpy as np

H = 96
W = 96
NIMG = 3
NCORES = 8
EPS = 1e-10
EPS1 = 2e-6
NSC = 16
BPS = 8            # bins per superchunk per half
BLK = 100          # per-bin block: 4 pads + 96 w
HCH = 5 + BPS * BLK + 2   # half-chunk cols: 5 lead + 800 + 2 trail = 807
PAD = 1e4          # xrep pad: sigmoid(10*PAD - anything) == 1 -> k == 0

_CACHE = {}


def _register_dve_ops():
    import concourse.dve_ops as dve_ops
    from concourse.dve_ops import DveOp
    from concourse.dve_spec import AluOp, One, Spec, Src0, Src1, scan

    def register(op):
        if op.name not in dve_ops._SUB_OPCODE_FOR_NAME:
            dve_ops.OPS.append(op)
            dve_ops._SUB_OPCODE_FOR_NAME[op.name] = (
                dve_ops._CUSTOM_DVE_ROW_BASE + len(dve_ops.OPS) - 1
            )
        else:
            op = next(o for o in dve_ops.OPS if o.name == op.name)
        return op

    win5 = register(DveOp(
        "WIN5K_ANT",
        Spec(body=scan(AluOp.ADD, Src0 * (One - Src0) - Src1 * (One - Src1)),
             reference=lambda in0, in1, c0, c1, c2: np.cumsum(
                 in0.astype(np.float32) * (1 - in0.astype(np.float32))
                 - in1.astype(np.float32) * (1 - in1.astype(np.float32)),
                 axis=-1, dtype=np.float32)),
        subdim=False,
        perf_en={"v3": True, "v4": True},
        uops_sha={"v3": "9d91f28b1ae18abb", "v4": "1425a9f273284709"}))

    return win5


def _emit_kernel(nc, tc, ctx, ins, outs, win5):
    from concourse import mybir

    f32 = mybir.dt.float32
    f16 = mybir.dt.float16
    i32 = mybir.dt.int32
    AF = mybir.ActivationFunctionType
    OP = mybir.AluOpType

    x_d, xrep_d, band_d = ins
    (ent_d,) = outs
    NW = NIMG * W

    consts = ctx.enter_context(tc.tile_pool(name="consts", bufs=1))
    sm = ctx.enter_context(tc.tile_pool(name="sm", bufs=1))
    vpool = ctx.enter_context(tc.tile_pool(name="vp", bufs=5))
    a5pool = ctx.enter_context(tc.tile_pool(name="a5p", bufs=1))
    lpool = ctx.enter_context(tc.tile_pool(name="lp", bufs=3))
    epool = ctx.enter_context(tc.tile_pool(name="ep", bufs=6))
    ppsum = ctx.enter_context(tc.tile_pool(name="pps", bufs=4, space="PSUM"))

    # ---- constants / inputs ----
    band_sb = consts.tile([H, H], f16)
    xrep = consts.tile([H, NIMG, 2 * HCH], f32)
    nc.gpsimd.dma_start(xrep[:, 0, 0:538], xrep_d[0][:, 0:538])
    nc.gpsimd.dma_start(xrep[:, 0, 538:1076], xrep_d[0][:, 538:1076])
    nc.gpsimd.dma_start(xrep[:, 0, 1076:1614], xrep_d[0][:, 1076:1614])
    nc.gpsimd.dma_start(band_sb[:], band_d[:])
    xall = consts.tile([H, NW], f32)
    for i in range(NIMG):
        nc.gpsimd.dma_start(xall[:, i * W:(i + 1) * W], x_d[i])
    for i in range(1, NIMG):
        nc.gpsimd.dma_start(xrep[:, i, :], xrep_d[i])

    bias_tiles = {}

    def bias_ap(val):
        if val not in bias_tiles:
            t = consts.tile([H, 1], f32, tag=f"bias{val}")
            nc.vector.memset(t[:], val)
            bias_tiles[val] = t
        return bias_tiles[val][:]

    acc0 = consts.tile([H, 1536], f32)
    acc1 = consts.tile([H, 1536], f32)
    acc2 = consts.tile([H, 1536], f32)
    accs = [acc0, acc1, acc2]
    QL = sm.tile([H, NW], f32)

    # S path tiles (ops are emitted inside image 0's phases)
    shp = sm.tile([H, NIMG, W + 4], f32)
    nc.vector.memset(shp[:], 0.0)
    swin = sm.tile([H, NIMG, W], f32)
    sw_flat = swin[:].rearrange("p a b -> p (a b)")
    rtile = sm.tile([H, NW], f32)
    lns = sm.tile([H, NW], f32)   # filled during image-0 ln phase

    def emit_spath_A():
        # frac = x - rint(x) (DVE, early: only needs the x DMA)
        ni = sm.tile([H, NW], i32)
        nc.vector.tensor_copy(ni[:], xall[:])
        nf = sm.tile([H, NW], f32)
        nc.vector.tensor_copy(nf[:], ni[:])
        frac = sm.tile([H, NW], f32)
        nc.vector.tensor_tensor(frac[:], xall[:], nf[:], op=OP.subtract)
        return nf, frac

    def emit_spath_B(frac):
        vtap = sm.tile([H, 3, NW], f32)
        for oi, o in enumerate((-1, 0, 1)):
            nc.scalar.activation(
                vtap[:, oi, :], frac[:], AF.Sigmoid,
                scale=10.0, bias=bias_ap(float(-10 * o)))
        return vtap

    def emit_spath_C(nf, vtap):
        ktap = sm.tile([H, 3, NW], f32)
        nc.vector.tensor_tensor(ktap[:], vtap[:], vtap[:], op=OP.mult)
        nc.vector.tensor_tensor(ktap[:], vtap[:], ktap[:], op=OP.subtract)
        spix = sm.tile([H, NW], f32)
        nc.vector.tensor_copy(spix[:], ktap[:, 1, :])
        for oi, o in ((0, -1), (2, 1)):
            m = sm.tile([H, NW], f32, tag=f"m{o}", name=f"m{o}")
            if o < 0:
                nc.vector.tensor_scalar(m[:], nf[:], 1.0, None, op0=OP.is_ge)
            else:
                nc.vector.tensor_scalar(m[:], nf[:], 254.0, None, op0=OP.is_le)
            nc.vector.tensor_tensor(m[:], m[:], ktap[:, oi, :], op=OP.mult)
            nc.vector.tensor_tensor(spix[:], spix[:], m[:], op=OP.add)
        spix16 = sm.tile([H, NW], f16)
        nc.vector.tensor_copy(spix16[:], spix[:])
        ps_s = ppsum.tile([H, 1024], f32, tag="pt")
        nc.tensor.matmul(
            ps_s[:, 0:NW], band_sb[:], spix16[:], start=True, stop=True)
        return ps_s

    def emit_spath_D(ps_s):
        sh = sm.tile([H, NW], f32)
        nc.scalar.copy(sh[:], ps_s[:, 0:NW])
        for ii in range(NIMG):
            nc.vector.tensor_copy(shp[:, ii, 2:2 + W], sh[:, ii * W:(ii + 1) * W])
        nc.vector.tensor_tensor(
            swin[:], shp[:, :, 0:W], shp[:, :, 1:1 + W], op=OP.add)
        for j in (2, 3, 4):
            nc.vector.tensor_tensor(swin[:], swin[:], shp[:, :, j:j + W], op=OP.add)
        nc.vector.tensor_scalar(rtile[:], sw_flat, EPS, None, op0=OP.add)
        nc.vector.reciprocal(rtile[:], rtile[:])

    # =====================  main loop  =====================
    # per-image sigmoid bias tiles; images >0 gate on the previous image's
    # last Ln output so the scheduler cannot interleave sigmoids (sigmoid
    # table) into the Ln phase (natural_log table) and thrash table loads
    sbias = []
    for i in range(NIMG):
        row = []
        for sc in range(NSC):
            sb_t = consts.tile([H, 1], f32, tag=f"sb{i}_{sc}", name=f"sb{i}_{sc}")
            row.append(sb_t)
        sbias.append(row)
    for sc in range(NSC):
        nc.vector.memset(sbias[0][sc][:], float(-80 * sc))
    last_lt = [None]

    def emit_reduce(i):
        racc = accs[i][:].rearrange("p (g w) -> p w g", g=16)
        nc.vector.tensor_reduce(
            QL[:, i * W:(i + 1) * W], racc,
            axis=mybir.AxisListType.X, op=OP.add)

    leafpool = ctx.enter_context(tc.tile_pool(name="leafp", bufs=1))
    et_hold = [None]

    def emit_combines(i, lvs):
        c0 = leafpool.tile([H, 1536], f16, tag="c0", name="c0")
        c1 = leafpool.tile([H, 1536], f16, tag="c1", name="c1")
        c2 = leafpool.tile([H, 1536], f16, tag="c2", name="c2")
        nc.vector.tensor_tensor(c0[:], lvs[0][:], lvs[1][:], op=OP.add)
        nc.vector.tensor_tensor(c1[:], lvs[2][:], lvs[3][:], op=OP.add)
        nc.vector.tensor_tensor(c2[:], lvs[4][:], lvs[5][:], op=OP.add)
        nc.vector.tensor_tensor(c0[:], c0[:], c1[:], op=OP.add)
        nc.vector.tensor_tensor(c1[:], lvs[6][:], lvs[7][:], op=OP.add)
        nc.vector.tensor_tensor(c1[:], c1[:], c2[:], op=OP.add)
        nc.vector.tensor_tensor(accs[i][:], c0[:], c1[:], op=OP.add)

    nf_frac = [None]
    vtap_h = [None]
    ps_s_h = [None]
    for i in range(NIMG):
        acc = accs[i]
        # ---- sigmoid phase: all 16 superchunks' front ends ----
        if i == 0:
            nf_frac[0] = emit_spath_A()
        a5s = []
        vt_last = None
        for sc in range(NSC):
            vt = vpool.tile([H, 2 * HCH], f16, tag="v")
            nc.scalar.activation(
                vt[:], xrep[:, i, :], AF.Sigmoid,
                scale=10.0, bias=sbias[i][sc][:])
            a5 = a5pool.tile([H, 2 * HCH], f16, tag=f"a5_{sc}")
            nc.vector._custom_dve(
                win5, out=a5[:, 0:2 * HCH - 5], in0=vt[:, 5:2 * HCH],
                in1=vt[:, 0:2 * HCH - 5])
            a5s.append(a5)
            vt_last = vt

        if i == 0:
            vtap_h[0] = emit_spath_B(nf_frac[0][1])
            ps_s_h[0] = emit_spath_C(nf_frac[0][0], vtap_h[0])
            emit_spath_D(ps_s_h[0])
        # gate this image's Ln phase on its own sigmoid phase (table order)
        ebias = consts.tile([H, 1], f32, tag=f"eb{i}", name=f"eb{i}")
        nc.vector.tensor_scalar(
            ebias[:], vt_last[:, 0:1], 0.0, EPS1, op0=OP.mult, op1=OP.add)

        leaves = []
        # ---- ln phase: half-superchunk granularity, 4-deep PSUM ----
        if i == 0:
            nc.scalar.activation(lns[:], sw_flat, AF.Ln, bias=bias_ap(EPS))

        def emit_band_h(j):
            sc, hf = j // 2, j % 2
            pt = ppsum.tile([H, 1024], f32, tag="pt")
            for pp in range(2):
                off = hf * HCH + 400 * pp + 6
                mv = a5s[sc][:, off:off + 400] \
                    .rearrange("p (b z) -> p b z", z=BLK)[:, :, 0:96]
                nc.tensor.matmul(
                    pt[:, 512 * pp:512 * pp + 384],
                    band_sb[:], mv, start=True, stop=True)
            return pt

        lt_cur = et_cur = None

        def emit_back_h(j, pt):
            nonlocal lt_cur, et_cur
            sc, hf = j // 2, j % 2
            if hf == 0:
                lt_cur = lpool.tile([H, 1536], f16, tag="l")
                et_cur = epool.tile([H, 1536], f16, tag="e")
            qv = pt[:].rearrange("p (a b) -> p a b", b=512)[:, :, 0:384]
            lv = lt_cur[:, hf * 768:(hf + 1) * 768] \
                .rearrange("p (a b) -> p a b", b=384)
            nc.scalar.activation(lv, qv, AF.Ln, bias=ebias[:])
            nc.vector.tensor_tensor(
                et_cur[:, hf * 768:(hf + 1) * 768]
                .rearrange("p (a b) -> p a b", b=384), qv, lv, op=OP.mult)
            if hf == 1:
                # Pool does only the 8 pair-leaves, inside the ln phase
                # (it cannot absorb a full 16-add chain and any spill into
                # the sigmoid phase halves both Pool and DVE throughput)
                if sc % 2 == 0:
                    et_hold[0] = et_cur
                else:
                    leaf = leafpool.tile([H, 1536], f16,
                                         tag=f"leaf{sc // 2}",
                                         name=f"leaf{sc // 2}")
                    nc.gpsimd.tensor_tensor(
                        leaf[:], et_hold[0][:], et_cur[:], op=OP.add)
                    leaves.append(leaf)
                if sc == NSC - 1 and i + 1 < NIMG:
                    last_lt[0] = lt_cur

        NH = 2 * NSC
        pts = {j: emit_band_h(j) for j in range(3)}
        for j in range(NH):
            if j + 3 < NH:
                pts[j + 3] = emit_band_h(j + 3)
            emit_back_h(j, pts.pop(j))

        if i + 1 < NIMG:
            for sc in range(NSC):
                nc.vector.tensor_scalar(
                    sbias[i + 1][sc][:], last_lt[0][:, 0:1], 0.0,
                    float(-80 * sc), op0=OP.mult, op1=OP.add)
        prev_leaves = leaves
    # Tail ops: manual scheduler floor so the list scheduler cannot park
    # them early in the in-order DVE queue.
    ctx.enter_context(tc.tile_wait_until(1.0))
    emit_combines(NIMG - 1, prev_leaves)
    emit_reduce(NIMG - 1)

    # E = lnS - r*T
    ent = sm.tile([H, NW], f32)
    nc.vector.tensor_tensor(ent[:], rtile[:], QL[:], op=OP.mult)
    nc.vector.tensor_tensor(ent[:], lns[:], ent[:], op=OP.subtract)
    for i in range(NIMG):
        nc.sync.dma_start(ent_d[i], ent[:, i * W:(i + 1) * W])


def _get_compiled():
    if "nc" in _CACHE:
        return _CACHE["nc"]
    from contextlib import ExitStack

    import concourse.tile as tile
    from concourse import bacc, mybir

    win5 = _register_dve_ops()

    f32 = mybir.dt.float32
    f16 = mybir.dt.float16
    nc = bacc.Bacc("TRN2", target_bir_lowering=False, debug=False)
    x_d = nc.dram_tensor("x_sh", [NIMG, H, W], f32, kind="ExternalInput").ap()
    xrep_d = nc.dram_tensor(
        "xrep", [NIMG, H, 2 * HCH], f32, kind="ExternalInput").ap()
    band_d = nc.dram_tensor("band16", [H, H], f16, kind="ExternalInput").ap()
    ent_d = nc.dram_tensor("ent", [NIMG, H, W], f32, kind="ExternalOutput").ap()

    with tile.TileContext(nc) as tc:
        with ExitStack() as ctx:
            _emit_kernel(nc, tc, ctx, (x_d, xrep_d, band_d), (ent_d,), win5)
    nc.compile()
    _CACHE["nc"] = nc
    return nc


def _build_xrep(imgs):
    """imgs: [N, 96, 96] f32 -> [N, 96, 1614] f32 padded x-b map."""
    n = imgs.shape[0]
    out = np.full((n, H, 2 * HCH), PAD, dtype=np.float32)
    # cols: half*807 + 5 + bl*100 + 4 + w  <-  x - bl - 128*half
    for half in range(2):
        for bl in range(BPS):
            c0 = half * HCH + 5 + bl * BLK + 4
            out[:, :, c0:c0 + W] = imgs - (bl + 128 * half)
    return out


def make_in_maps(x):
    """x: full [8, 3, 96, 96] -> list of 8 per-core input dicts."""
    x = np.ascontiguousarray(np.asarray(x, dtype=np.float32))
    imgs = x.reshape(NCORES * NIMG, H, W)
    hh = np.arange(H)
    band = (np.abs(hh[:, None] - hh[None, :]) <= 2).astype(np.float16)
    xrep = _build_xrep(imgs)
    in_maps = []
    for c in range(NCORES):
        sh = np.ascontiguousarray(imgs[c * NIMG:(c + 1) * NIMG])
        in_maps.append(
            {
                "x_sh": sh,
                "xrep": np.ascontiguousarray(xrep[c * NIMG:(c + 1) * NIMG]),
                "band16": band,
            }
        )
    return in_maps


def kernel(x):
    """Full inputs in, full outputs out. x: [8, 3, 96, 96] f32."""
    from concourse.bass_utils import run_bass_kernel_spmd

    nc = _get_compiled()
    in_maps = make_in_maps(x)
    res = run_bass_kernel_spmd(nc, in_maps, list(range(NCORES)))
    out = np.stack([res.results[c]["ent"] for c in range(NCORES)])
    return out.reshape(8, 3, H, W).astype(np.float32)
